# revision 12
# baseline (speedup 1.0000x reference)
"""Trainium2 Bass kernel for nn_AttentionDe_lm (conv-projected multi-head attention).

Strategy: pure data-parallel over batch B=8 -> one batch element per NeuronCore.
Per core, everything is formulated as PE matmuls in a channels-on-partitions
layout [C, H*W]:

  - depthwise 3x3 convs   -> 9 PSUM-accumulated matmuls with diagonal weight
                             matrices; zero padding is realized by clipping the
                             per-tap input windows (center tap issued first so
                             the accumulation covers the whole tile)
  - pointwise 1x1 convs   -> plain matmuls (weights pre-transposed and
                             head-major-permuted on the host)
  - attention             -> computed transposed: T = S^T tiles [j, i] so that
                             QK^T needs no transposes at all; the two heads of a
                             partition chunk run concurrently in the PE array
                             via row tile_position packing; exp on ScalarE with
                             the attention scale fused in (inputs are tiny, so
                             no max-subtraction is needed); AV consumes E with V
                             extended by a ones column (M=65) so the softmax
                             denominators emerge from the same matmul chain
  - softmax normalization -> reciprocal + partition-broadcast (via a DRAM
                             bounce) + one fused multiply that writes the bf16
                             input of the output depthwise conv

Matmuls run in fp32r (full PE rate at N=512); attention probabilities and the
output-side depthwise conv run in bf16.
"""

import sys

sys.path.insert(0, "/opt/trn_rl_repo")

import numpy as np
import concourse.bass as bass
import concourse.tile as tile
from concourse import mybir, bass_utils
from concourse.vector_clock import ScopedClock, VectorClock

# ---------------------------------------------------------------------------
# TileContext adapted to a walrus build that allows at most ONE sync-wait per
# instruction: hoist extra waits onto EventSemaphore instructions, and replace
# the multi-wait final Drain with per-sem single-wait SP no-ops.
# ---------------------------------------------------------------------------

_ev_counter = [0]


class SplitDrainTileContext(tile.TileContext):
    def _split_multi_waits(self):
        f = self.nc.cur_f
        assert f is not None
        for bb in f.blocks[self.starting_block_idx :]:
            out = []
            changed = False
            for inst in list(bb.instructions):
                si = inst.sync_info
                if si is not None and len(si.on_wait) > 1:
                    changed = True
                    waits = list(si.on_wait)
                    for w in waits[:-1]:
                        _ev_counter[0] += 1
                        ev = mybir.InstEventSemaphore(name=f"IW-{_ev_counter[0]}")
                        ev.engine = inst.engine
                        ev.sync_info = mybir.SyncInfo(on_wait=[w], on_update=[])
                        self.nc.register_instruction(ev, overwrite=True)
                        out.append(ev)
                    inst.sync_info = mybir.SyncInfo(
                        on_wait=[waits[-1]], on_update=list(si.on_update)
                    )
                out.append(inst)
            if changed:
                bb.instructions = out

    def _drain_and_barrier(self, tick_clock, wait_clock):
        gvec = list(tick_clock.global_clock)
        nprocs = len(gvec)
        for p, t in enumerate(gvec):
            if t <= 0:
                continue
            vec = [0] * nprocs
            vec[p] = t
            ev = self.nc.sync.nop()
            wait_clock.add_sem_waits(ev.ins, ScopedClock({None: VectorClock(vec)}))
        self.nc.sync.drain()
        self.nc.all_engine_barrier()
        assert self.sems is not None
        popped = self.nc._tile_sem_poison_stack.pop()
        assert popped is self._sem_poison
        self.nc.clear_and_free_semaphores(list(self.sems.allocated().values()))
        self.nc.all_engine_barrier()
        self._split_multi_waits()


# ---------------------------------------------------------------------------
# Problem constants (hardcoded per the harness contract)
# ---------------------------------------------------------------------------

B, C, H, W = 8, 256, 32, 32
N = H * W                      # 1024 spatial positions
HEADS, D = 8, 64
INNER = HEADS * D              # 512
SCALE = D ** -0.5
P = 128
N_CORES = 8

f32 = mybir.dt.float32
f32r = mybir.dt.float32r
bf16 = mybir.dt.bfloat16
Exp = mybir.ActivationFunctionType.Exp

# center tap first: its full window makes start=True cover the whole psum tile
TAP_ORDER = [4, 0, 1, 2, 3, 5, 6, 7, 8]


def _bcast_ap(dram_tile, parts):
    """Partition-broadcast view of a [1, F] DRAM tile."""
    return bass.AP(
        tensor=dram_tile.tensor,
        offset=dram_tile.offset,
        ap=[[0, parts]] + list(dram_tile.ap[1:]),
    )


def _dw3x3(nc, ps_pool, dst_sb, src3d, diag, slot, evac):
    """Depthwise 3x3 (pad=1) on a width-padded [128, 32, 34] image via 9
    PSUM-accumulated diagonal matmuls. The center tap goes first so its full
    window makes start=True cover the whole psum tile; vertical padding is
    realized by clipping output rows.

    dst_sb: [128, 1024] SBUF tile; evac: copy fn (psum_ap, dst_ap) -> None.
    """
    for half in range(2):
        r0 = half * 16
        acc = ps_pool.tile([P, 16, W], f32, tag="mm")
        for i, t in enumerate(TAP_ORDER):
            oy, dx = t // 3 - 1, t % 3
            rs, re = max(r0, -oy), min(r0 + 16, H - oy)
            nc.tensor.matmul(
                acc[:, rs - r0 : re - r0, :],
                diag[:, slot, t, :],
                src3d[:, rs + oy : re + oy, dx : dx + W],
                start=(i == 0), stop=(i == 8),
            )
        evac(acc[:].rearrange("p a b -> p (a b)"),
             dst_sb[:, half * 512 : (half + 1) * 512])


def _build_nc():
    nc = bass.Bass("TRN2", target_bir_lowering=False, debug=False, enable_asserts=True)

    # Per-core inputs (one batch element) + replicated preprocessed weights.
    q_ap = nc.dram_tensor("q", (C, H * (W + 2)), f32r, kind="ExternalInput").ap()
    x_ap = nc.dram_tensor("x", (C, H * (W + 2)), f32r, kind="ExternalInput").ap()
    ident_ap = nc.dram_tensor("ident", (P, P), f32, kind="ExternalInput").ap()
    dw9qx_ap = nc.dram_tensor("dw9qx", (P, 4, 9), f32, kind="ExternalInput").ap()
    dw9o_ap = nc.dram_tensor("dw9o", (P, 4, 9), f32, kind="ExternalInput").ap()
    qpw_ap = nc.dram_tensor("qpw", (P, 2, INNER), f32r, kind="ExternalInput").ap()
    kpw_ap = nc.dram_tensor("kpw", (P, 2, INNER), f32r, kind="ExternalInput").ap()
    vpw_ap = nc.dram_tensor("vpw", (P, 2, INNER), f32r, kind="ExternalInput").ap()
    opw_ap = nc.dram_tensor("opw", (P, 4, C), f32r, kind="ExternalInput").ap()
    out_ap = nc.dram_tensor("out", (C, N), f32, kind="ExternalOutput").ap()

    with SplitDrainTileContext(nc) as tc:
        with (
            tc.tile_pool(name="const", bufs=1) as const,
            tc.tile_pool(name="persist", bufs=1) as persist,
            tc.tile_pool(name="epool", bufs=10) as epool,
            tc.tile_pool(name="norm", bufs=3) as norm,
            tc.tile_pool(name="dram", bufs=4, space="DRAM") as drp,
            tc.tile_pool(name="ps_t", bufs=2, space="PSUM") as ps_t,
            tc.tile_pool(name="ps_o", bufs=2, space="PSUM") as ps_o,
            tc.tile_pool(name="ps_m", bufs=2, space="PSUM") as ps_m,
        ):
            # ---------------- constant weights ----------------
            # (depthwise diagonals are expanded on-device from compact
            #  [128, 4, 9] host arrays: diag = identity * w, per partition)
            ident = const.tile([P, P], f32)
            nc.sync.dma_start(ident[:], ident_ap[:])
            dw9qx = const.tile([P, 4, 9], f32)
            nc.sync.dma_start(dw9qx[:], dw9qx_ap[:])
            dw9o = const.tile([P, 4, 9], f32)
            nc.scalar.dma_start(dw9o[:], dw9o_ap[:])
            dgo = const.tile([P, 4, 9, P], bf16)
            qpw = const.tile([P, 2, INNER], f32r)
            nc.scalar.dma_start(qpw[:], qpw_ap[:])
            kpw = const.tile([P, 2, INNER], f32r)
            nc.scalar.dma_start(kpw[:], kpw_ap[:])
            vpw = const.tile([P, 2, INNER], f32r)
            nc.scalar.dma_start(vpw[:], vpw_ap[:])
            opw = const.tile([P, 4, C], f32r)
            nc.scalar.dma_start(opw[:], opw_ap[:])

            # persistent activations
            Q = [persist.tile([P, N], f32r, name=f"Q{i}") for i in range(4)]
            K = [persist.tile([P, N], f32r, name=f"K{i}") for i in range(4)]
            Vp = [persist.tile([P, HEADS, D + 1], bf16, name=f"Vp{i}")
                  for i in range(8)]
            o3d = [persist.tile([P, H, W + 2], bf16, name=f"o3d{i}") for i in range(4)]
            od = [persist.tile([P, N], f32r, name=f"od{i}") for i in range(4)]
            for ck in range(4):
                nc.scalar.memzero(o3d[ck][:])

            def evac_act(src, dst):
                nc.scalar.copy(dst, src)

            def evac_dve(src, dst):
                nc.vector.tensor_copy(dst, src)

            # ---------------- phase A: input dw convs + projections ---------
            with tc.tile_pool(name="phaseA", bufs=1) as pa:
                dgqx = pa.tile([P, 4, 9, P], f32r)
                for s in range(4):
                    for t in range(9):
                        nc.vector.tensor_scalar_mul(
                            dgqx[:, s, t, :], ident[:], dw9qx[:, s, t : t + 1]
                        )
                raws = []
                dma_engines = [nc.sync, nc.scalar, nc.gpsimd, nc.sync]
                for src_ap, nm in ((q_ap, "qr"), (x_ap, "xr")):
                    for ck in range(2):
                        raw = pa.tile([P, H, W + 2], f32r, name=f"{nm}{ck}")
                        dma_engines[len(raws)].dma_start(
                            raw[:],
                            src_ap[ck * P : (ck + 1) * P, :].rearrange(
                                "p (a b) -> p a b", b=W + 2
                            ),
                        )
                        raws.append(raw)

                qd = [pa.tile([P, N], f32r, name=f"qd{i}") for i in range(2)]
                xd = [pa.tile([P, N], f32r, name=f"xd{i}") for i in range(2)]
                for s in range(4):                  # slots: q0,q1,x0,x1
                    dst = qd[s] if s < 2 else xd[s - 2]
                    _dw3x3(nc, ps_m, dst, raws[s], dgqx, s, evac_act)

                for oc in range(4):
                    for nh in range(2):
                        acc = ps_m.tile([P, 512], f32, tag="mm")
                        for kc in range(2):
                            nc.tensor.matmul(
                                acc[:], qpw[:, kc, oc * P : (oc + 1) * P],
                                qd[kc][:, nh * 512 : (nh + 1) * 512],
                                start=(kc == 0), stop=(kc == 1),
                            )
                        nc.vector.tensor_copy(
                            Q[oc][:, nh * 512 : (nh + 1) * 512], acc[:]
                        )
                for oc in range(4):
                    for nh in range(2):
                        acc = ps_m.tile([P, 512], f32, tag="mm")
                        for kc in range(2):
                            nc.tensor.matmul(
                                acc[:], kpw[:, kc, oc * P : (oc + 1) * P],
                                xd[kc][:, nh * 512 : (nh + 1) * 512],
                                start=(kc == 0), stop=(kc == 1),
                            )
                        nc.vector.tensor_copy(
                            K[oc][:, nh * 512 : (nh + 1) * 512], acc[:]
                        )
                # V transposed: V^T[j, v_ch]; per j-chunk [128, 8, 65]
                # (per head: 64 value dims + ones column for softmax sums)
                for jc in range(8):
                    nc.vector.memset(Vp[jc][:], 1.0)
                    acc = ps_m.tile([P, 512], f32, tag="mm")
                    for kc in range(2):
                        nc.tensor.matmul(
                            acc[:], xd[kc][:, jc * P : (jc + 1) * P],
                            vpw[:, kc, :], start=(kc == 0), stop=(kc == 1),
                        )
                    nc.vector.tensor_copy(
                        Vp[jc][:, :, 0:D],
                        acc[:].rearrange("p (h d) -> p h d", d=D),
                    )

            for s in range(4):
                for t in range(9):
                    nc.vector.tensor_scalar_mul(
                        dgo[:, s, t, :], ident[:], dw9o[:, s, t : t + 1]
                    )

            # ---------------- attention + output path ----------------------
            for p in range(4):                       # head pair (= chunk)
                for ih in range(2):                  # query half (i block)
                    Otiles = [ps_o.tile([P, 512], f32, tag="O", name=f"O{p}{ih}{k}")
                              for k in range(2)]
                    for jc in range(8):
                        T = ps_t.tile([P, 1024], f32, tag="T")
                        nc.tensor.matmul(
                            T[:, 0:512],
                            K[p][0:64, jc * P : (jc + 1) * P],
                            Q[p][0:64, ih * 512 : (ih + 1) * 512],
                            start=True, stop=True, tile_position=(0, 0),
                        )
                        nc.tensor.matmul(
                            T[:, 512:1024],
                            K[p][64:128, jc * P : (jc + 1) * P],
                            Q[p][64:128, ih * 512 : (ih + 1) * 512],
                            start=True, stop=True, tile_position=(64, 0),
                        )
                        E = epool.tile([P, 1024], bf16, tag="E")
                        nc.scalar.activation(E[:], T[:], Exp, scale=SCALE)
                        for hs in range(2):
                            nc.tensor.matmul(
                                Otiles[hs][0:65, :],
                                Vp[jc][:, 2 * p + hs, :],
                                E[:, hs * 512 : (hs + 1) * 512],
                                start=(jc == 0), stop=(jc == 7),
                            )

                    for hs in range(2):              # head within the pair
                        O = Otiles[hs]
                        # softmax denominators sit in row 64; normalize rows
                        # 0..63 into the bf16 out-conv input.
                        rc = norm.tile([P, 512], f32, tag="rc")
                        nc.vector.reciprocal(rc[0:1, :], O[64:65, :])
                        dsc = drp.tile([1, 512], f32, tag="dsc")
                        nc.sync.dma_start(dsc[:], rc[0:1, :])
                        bc = norm.tile([P, 512], f32, tag="bc")
                        nc.sync.dma_start(bc[0:64, :], _bcast_ap(dsc, 64))
                        nc.vector.tensor_mul(
                            o3d[p][64 * hs : 64 * hs + 64,
                                   16 * ih : 16 * ih + 16, 1 : 1 + W],
                            O[0:64, :].rearrange("p (a b) -> p a b", b=W),
                            bc[0:64, :].rearrange("p (a b) -> p a b", b=W),
                        )

                # output depthwise for the previous chunk (emitted here so
                # this pair's QK stream sits ahead of it in the PE queue)
                if p > 0:
                    _dw3x3(nc, ps_m, od[p - 1], o3d[p - 1], dgo, p - 1, evac_dve)
            _dw3x3(nc, ps_m, od[3], o3d[3], dgo, 3, evac_dve)

            # ---------------- final pointwise + store -----------------------
            for oc in range(2):
                out_sb = persist.tile([P, N], f32, name=f"outsb{oc}")
                for nh in range(2):
                    acc = ps_m.tile([P, 512], f32, tag="mm")
                    for kc in range(4):
                        nc.tensor.matmul(
                            acc[:], opw[:, kc, oc * P : (oc + 1) * P],
                            od[kc][:, nh * 512 : (nh + 1) * 512],
                            start=(kc == 0), stop=(kc == 3),
                        )
                    nc.vector.tensor_copy(
                        out_sb[:, nh * 512 : (nh + 1) * 512], acc[:]
                    )
                    nc.sync.dma_start(
                        out_ap[oc * P : (oc + 1) * P, nh * 512 : (nh + 1) * 512],
                        out_sb[:, nh * 512 : (nh + 1) * 512],
                    )

    return nc


_NC_CACHE = {}
LAST_RESULTS = None


def _get_nc():
    if "nc" not in _NC_CACHE:
        _NC_CACHE["nc"] = _build_nc()
    return _NC_CACHE["nc"]


def _prep_weights(q_dw, q_pw, kv_dw, kv_pw, out_dw, out_pw):
    m = np.arange(INNER)
    perm = (m % D) * HEADS + (m // D)        # head-major -> original channel

    def pw_T(w):                              # [out, in] -> SBUF [128, in/128, out]
        wT = np.ascontiguousarray(w.T)        # [in, out]
        kchunks = wT.shape[0] // P
        return np.ascontiguousarray(
            wT.reshape(kchunks, P, wT.shape[1]).transpose(1, 0, 2)
        )

    qpw = pw_T(q_pw.reshape(INNER, C)[perm])
    kpw = pw_T(kv_pw.reshape(2 * INNER, C)[:INNER][perm])
    vpw = pw_T(kv_pw.reshape(2 * INNER, C)[INNER:][perm])
    opw = pw_T(out_pw.reshape(C, INNER)[:, perm])     # -> lhsT [128, 4, 256]

    qdw = q_dw.reshape(C, 9)
    xdw = kv_dw.reshape(C, 9)
    odw = out_dw.reshape(INNER, 9)[perm]

    dw9qx = np.stack([qdw[0:P], qdw[P:2 * P], xdw[0:P], xdw[P:2 * P]], axis=1)
    dw9o = np.stack([odw[0:P], odw[P:2 * P], odw[2 * P:3 * P], odw[3 * P:4 * P]],
                    axis=1)

    return {
        "ident": np.eye(P, dtype=np.float32),
        "dw9qx": np.ascontiguousarray(dw9qx),
        "dw9o": np.ascontiguousarray(dw9o),
        "qpw": qpw,
        "kpw": kpw,
        "vpw": vpw,
        "opw": opw,
    }


def kernel(q, x, q_dw, q_pw, kv_dw, kv_pw, out_dw, out_pw):
    global LAST_RESULTS
    q = np.asarray(q, np.float32)
    x = np.asarray(x, np.float32)
    weights = _prep_weights(
        np.asarray(q_dw, np.float32), np.asarray(q_pw, np.float32),
        np.asarray(kv_dw, np.float32), np.asarray(kv_pw, np.float32),
        np.asarray(out_dw, np.float32), np.asarray(out_pw, np.float32),
    )
    in_maps = []
    for b in range(N_CORES):
        qp = np.zeros((C, H, W + 2), np.float32)
        qp[:, :, 1 : 1 + W] = q[b].reshape(C, H, W)
        xp = np.zeros((C, H, W + 2), np.float32)
        xp[:, :, 1 : 1 + W] = x[b].reshape(C, H, W)
        m = {"q": qp.reshape(C, -1), "x": xp.reshape(C, -1)}
        m.update(weights)
        in_maps.append(m)

    nc = _get_nc()
    res = bass_utils.run_bass_kernel_spmd(nc, in_maps, core_ids=list(range(N_CORES)))
    LAST_RESULTS = res
    out = np.stack([res.results[b]["out"].reshape(C, H, W) for b in range(N_CORES)])
    return out.astype(np.float32)


# revision 19
# speedup vs baseline: 529.3480x; 529.3480x over previous
"""Trainium2 Bass kernel for nn_AttentionDe_lm (conv-projected multi-head attention).

Strategy: pure data-parallel over batch B=8 -> one batch element per NeuronCore.
Per core, everything is formulated as PE matmuls in a channels-on-partitions
layout [C, H*W]:

  - depthwise 3x3 convs   -> 9 PSUM-accumulated matmuls with diagonal weight
                             matrices; zero padding is realized by clipping the
                             per-tap input windows (center tap issued first so
                             the accumulation covers the whole tile)
  - pointwise 1x1 convs   -> plain matmuls (weights pre-transposed and
                             head-major-permuted on the host)
  - attention             -> computed transposed: T = S^T tiles [j, i] so that
                             QK^T needs no transposes at all; the two heads of a
                             partition chunk run concurrently in the PE array
                             via row tile_position packing; exp on ScalarE with
                             the attention scale fused in (inputs are tiny, so
                             no max-subtraction is needed); AV consumes E with V
                             extended by a ones column (M=65) so the softmax
                             denominators emerge from the same matmul chain
  - softmax normalization -> reciprocal + partition-broadcast (via a DRAM
                             bounce) + one fused multiply that writes the bf16
                             input of the output depthwise conv

Matmuls run in fp32r (full PE rate at N=512); attention probabilities and the
output-side depthwise conv run in bf16.
"""

import sys

sys.path.insert(0, "/opt/trn_rl_repo")

import numpy as np
import concourse.bass as bass
import concourse.tile as tile
from concourse import mybir, bass_utils
from concourse.vector_clock import ScopedClock, VectorClock

# ---------------------------------------------------------------------------
# TileContext adapted to a walrus build that allows at most ONE sync-wait per
# instruction: hoist extra waits onto EventSemaphore instructions, and replace
# the multi-wait final Drain with per-sem single-wait SP no-ops.
# ---------------------------------------------------------------------------

_ev_counter = [0]


class SplitDrainTileContext(tile.TileContext):
    def _split_multi_waits(self):
        f = self.nc.cur_f
        assert f is not None
        for bb in f.blocks[self.starting_block_idx :]:
            out = []
            changed = False
            for inst in list(bb.instructions):
                si = inst.sync_info
                if si is not None and len(si.on_wait) > 1:
                    changed = True
                    waits = list(si.on_wait)
                    for w in waits[:-1]:
                        _ev_counter[0] += 1
                        ev = mybir.InstEventSemaphore(name=f"IW-{_ev_counter[0]}")
                        ev.engine = inst.engine
                        ev.sync_info = mybir.SyncInfo(on_wait=[w], on_update=[])
                        self.nc.register_instruction(ev, overwrite=True)
                        out.append(ev)
                    inst.sync_info = mybir.SyncInfo(
                        on_wait=[waits[-1]], on_update=list(si.on_update)
                    )
                out.append(inst)
            if changed:
                bb.instructions = out

    def _drain_and_barrier(self, tick_clock, wait_clock):
        gvec = list(tick_clock.global_clock)
        nprocs = len(gvec)
        for p, t in enumerate(gvec):
            if t <= 0:
                continue
            vec = [0] * nprocs
            vec[p] = t
            ev = self.nc.sync.nop()
            wait_clock.add_sem_waits(ev.ins, ScopedClock({None: VectorClock(vec)}))
        self.nc.sync.drain()
        self.nc.all_engine_barrier()
        assert self.sems is not None
        popped = self.nc._tile_sem_poison_stack.pop()
        assert popped is self._sem_poison
        self.nc.clear_and_free_semaphores(list(self.sems.allocated().values()))
        self.nc.all_engine_barrier()
        self._split_multi_waits()


# ---------------------------------------------------------------------------
# Problem constants (hardcoded per the harness contract)
# ---------------------------------------------------------------------------

B, C, H, W = 8, 256, 32, 32
N = H * W                      # 1024 spatial positions
HEADS, D = 8, 64
INNER = HEADS * D              # 512
SCALE = D ** -0.5
P = 128
N_CORES = 8

f32 = mybir.dt.float32
f32r = mybir.dt.float32r
bf16 = mybir.dt.bfloat16
Exp = mybir.ActivationFunctionType.Exp

# center tap first: its full window makes start=True cover the whole psum tile
TAP_ORDER = [4, 0, 1, 2, 3, 5, 6, 7, 8]


def _bcast_ap(dram_tile, parts):
    """Partition-broadcast view of a [1, F] DRAM tile."""
    return bass.AP(
        tensor=dram_tile.tensor,
        offset=dram_tile.offset,
        ap=[[0, parts]] + list(dram_tile.ap[1:]),
    )


def _dw3x3(nc, ps_pool, dst_sb, src3d, diag, slot, evac):
    """Depthwise 3x3 (pad=1) on a width-padded [128, 32, 34] image via 9
    PSUM-accumulated diagonal matmuls. The center tap goes first so its full
    window makes start=True cover the whole psum tile; vertical padding is
    realized by clipping output rows.

    dst_sb: [128, 1024] SBUF tile; evac: copy fn (psum_ap, dst_ap) -> None.
    """
    for half in range(2):
        _dw3x3_half(nc, ps_pool, dst_sb, src3d, diag, slot, half, evac)


def _dw3x3_half(nc, ps_pool, dst_sb, src3d, diag, slot, half, evac):
    r0 = half * 16
    acc = ps_pool.tile([P, 16, W], f32, tag="mm")
    for i, t in enumerate(TAP_ORDER):
        oy, dx = t // 3 - 1, t % 3
        rs, re = max(r0, -oy), min(r0 + 16, H - oy)
        nc.tensor.matmul(
            acc[:, rs - r0 : re - r0, :],
            diag[:, slot, t, :],
            src3d[:, rs + oy : re + oy, dx : dx + W],
            start=(i == 0), stop=(i == 8),
        )
    evac(acc[:].rearrange("p a b -> p (a b)"),
         dst_sb[:, half * 512 : (half + 1) * 512])


def _build_nc():
    nc = bass.Bass("TRN2", target_bir_lowering=False, debug=False, enable_asserts=True)

    # Per-core inputs (one batch element) + replicated preprocessed weights.
    q_ap = nc.dram_tensor("q", (C, H * (W + 2)), f32r, kind="ExternalInput").ap()
    x_ap = nc.dram_tensor("x", (C, H * (W + 2)), f32r, kind="ExternalInput").ap()
    ident_ap = nc.dram_tensor("ident", (P, P), f32, kind="ExternalInput").ap()
    dw9qx_ap = nc.dram_tensor("dw9qx", (P, 4, 9), f32, kind="ExternalInput").ap()
    dw9o_ap = nc.dram_tensor("dw9o", (P, 4, 9), f32, kind="ExternalInput").ap()
    qpw_ap = nc.dram_tensor("qpw", (P, 2, INNER), f32r, kind="ExternalInput").ap()
    kpw_ap = nc.dram_tensor("kpw", (P, 2, INNER), f32r, kind="ExternalInput").ap()
    vpw_ap = nc.dram_tensor("vpw", (P, 2, INNER), f32r, kind="ExternalInput").ap()
    opw_ap = nc.dram_tensor("opw", (P, 4, C), f32r, kind="ExternalInput").ap()
    ones_ap = nc.dram_tensor("ones64", (1, 64), f32r, kind="ExternalInput").ap()
    out_ap = nc.dram_tensor("out", (C, N), f32, kind="ExternalOutput").ap()

    with SplitDrainTileContext(nc) as tc:
        with (
            tc.tile_pool(name="const", bufs=1) as const,
            tc.tile_pool(name="persist", bufs=1) as persist,
            tc.tile_pool(name="epool", bufs=10) as epool,
            tc.tile_pool(name="norm", bufs=3) as norm,
            tc.tile_pool(name="dram", bufs=4, space="DRAM") as drp,
            tc.tile_pool(name="ps_t", bufs=2, space="PSUM") as ps_t,
            tc.tile_pool(name="ps_o", bufs=2, space="PSUM") as ps_o,
            tc.tile_pool(name="ps_m", bufs=2, space="PSUM") as ps_m,
        ):
            # ---------------- constant weights ----------------
            # (depthwise diagonals are expanded on-device from compact
            #  [128, 4, 9] host arrays: diag = identity * w, per partition)
            ident = const.tile([P, P], f32)
            nc.sync.dma_start(ident[:], ident_ap[:])
            dw9qx = const.tile([P, 4, 9], f32)
            nc.sync.dma_start(dw9qx[:], dw9qx_ap[:])
            dw9o = const.tile([P, 4, 9], f32)
            nc.scalar.dma_start(dw9o[:], dw9o_ap[:])
            dgo = const.tile([P, 4, 9, P], bf16)
            qpw = const.tile([P, 2, INNER], f32r)
            nc.scalar.dma_start(qpw[:], qpw_ap[:])
            kpw = const.tile([P, 2, INNER], f32r)
            nc.scalar.dma_start(kpw[:], kpw_ap[:])
            vpw = const.tile([P, 2, INNER], f32r)
            nc.scalar.dma_start(vpw[:], vpw_ap[:])
            opw = const.tile([P, 4, C], f32r)
            nc.scalar.dma_start(opw[:], opw_ap[:])
            ones64 = const.tile([1, 64], f32r)
            nc.sync.dma_start(ones64[:], ones_ap[:])

            # persistent activations
            Q = [persist.tile([P, N], f32r, name=f"Q{i}") for i in range(4)]
            K = [persist.tile([P, N], f32r, name=f"K{i}") for i in range(4)]
            Vp = [persist.tile([P, HEADS, D + 1], bf16, name=f"Vp{i}")
                  for i in range(8)]
            o3d = [persist.tile([P, H, W + 2], bf16, name=f"o3d{i}") for i in range(4)]
            od = [persist.tile([P, N], f32r, name=f"od{i}") for i in range(4)]
            for ck in range(4):
                nc.scalar.memzero(o3d[ck][:])

            def evac_act(src, dst):
                nc.scalar.copy(dst, src)

            def evac_dve(src, dst):
                nc.vector.tensor_copy(dst, src)

            # ---------------- phase A: input dw convs + projections ---------
            with tc.tile_pool(name="phaseA", bufs=1) as pa:
                dgqx = pa.tile([P, 4, 9, P], f32r)
                for s in range(4):
                    for t in range(9):
                        nc.vector.tensor_scalar_mul(
                            dgqx[:, s, t, :], ident[:], dw9qx[:, s, t : t + 1]
                        )
                dma_engines = [nc.sync, nc.scalar, nc.gpsimd]
                di = 0
                rawmap = {}
                for src_ap, nm in ((x_ap, "xr"), (q_ap, "qr")):
                    for ck in range(2):
                        raw = pa.tile([P, H, W + 2], f32r, name=f"{nm}{ck}")
                        for hh in range(2):
                            dma_engines[di % 3].dma_start(
                                raw[64 * hh : 64 * hh + 64],
                                src_ap[ck * P + 64 * hh : ck * P + 64 * hh + 64, :]
                                .rearrange("p (a b) -> p a b", b=W + 2),
                            )
                            di += 1
                        rawmap[f"{nm}{ck}"] = raw
                raws = [rawmap["qr0"], rawmap["qr1"], rawmap["xr0"], rawmap["xr1"]]

                qd = [persist.tile([P, N], f32r, name=f"qd{i}") for i in range(2)]
                xd = [persist.tile([P, N], f32r, name=f"xd{i}") for i in range(2)]
                for s in (2, 3, 0, 1):              # x first: K/V unblock sooner
                    dst = qd[s] if s < 2 else xd[s - 2]
                    _dw3x3(nc, ps_m, dst, raws[s], dgqx, s, evac_act)

                def proj_qk(oc):
                    for w_sb, dsrc, dst in ((qpw, qd, Q), (kpw, xd, K)):
                        for nh in range(2):
                            acc = ps_m.tile([P, 512], f32, tag="mm")
                            for kc in range(2):
                                nc.tensor.matmul(
                                    acc[:], w_sb[:, kc, oc * P : (oc + 1) * P],
                                    dsrc[kc][:, nh * 512 : (nh + 1) * 512],
                                    start=(kc == 0), stop=(kc == 1),
                                )
                            nc.vector.tensor_copy(
                                dst[oc][:, nh * 512 : (nh + 1) * 512], acc[:]
                            )

                proj_qk(0)
                # V transposed: V^T[j, v_ch]; per j-chunk [128, 8, 65]
                # (per head: 64 value dims + ones column for softmax sums).
                # Emission is deferred into pair 0's jc loop so the first QK
                # tiles reach the PE sooner.
                for jc in range(8):
                    nc.vector.memset(Vp[jc][:], 1.0)

                for jc in range(8):
                    acc = ps_m.tile([P, 512], f32, tag="mm")
                    for kc in range(2):
                        nc.tensor.matmul(
                            acc[:], xd[kc][:, jc * P : (jc + 1) * P],
                            vpw[:, kc, :], start=(kc == 0), stop=(kc == 1),
                        )
                    nc.vector.tensor_copy(
                        Vp[jc][:, :, 0:D],
                        acc[:].rearrange("p (h d) -> p h d", d=D),
                    )

            for s in range(4):
                for t in range(9):
                    nc.vector.tensor_scalar_mul(
                        dgo[:, s, t, :], ident[:], dw9o[:, s, t : t + 1]
                    )

            # ---------------- attention + output path ----------------------
            for p in range(4):                       # head pair (= chunk)
                for ih in range(2):                  # query half (i block)
                    Otiles = [ps_o.tile([P, 512], f32, tag="O", name=f"O{p}{ih}{k}")
                              for k in range(2)]
                    for jc in range(8):
                        T = ps_t.tile([P, 1024], f32, tag="T")
                        nc.tensor.matmul(
                            T[:, 0:512],
                            K[p][0:64, jc * P : (jc + 1) * P],
                            Q[p][0:64, ih * 512 : (ih + 1) * 512],
                            start=True, stop=True, tile_position=(0, 0),
                        )
                        nc.tensor.matmul(
                            T[:, 512:1024],
                            K[p][64:128, jc * P : (jc + 1) * P],
                            Q[p][64:128, ih * 512 : (ih + 1) * 512],
                            start=True, stop=True, tile_position=(64, 0),
                        )
                        E = epool.tile([P, 1024], bf16, tag="E")
                        nc.scalar.activation(E[:], T[:], Exp, scale=SCALE)
                        for hs in range(2):
                            nc.tensor.matmul(
                                Otiles[hs][0:65, :],
                                Vp[jc][:, 2 * p + hs, :],
                                E[:, hs * 512 : (hs + 1) * 512],
                                start=(jc == 0), stop=(jc == 7),
                            )
                        if ih == 0 and p >= 1 and jc in (2, 4):
                            _dw3x3_half(nc, ps_m, od[p - 1], o3d[p - 1],
                                        dgo, p - 1, jc // 2 - 1, evac_dve)
                        if ih == 0 and p < 3 and jc == 6:
                            proj_qk(p + 1)
                    for hs in range(2):              # head within the pair
                        O = Otiles[hs]
                        # softmax denominators sit in row 64; normalize rows
                        # 0..63 into the bf16 out-conv input.
                        rc = norm.tile([P, 512], f32r, tag="rc")
                        with nc.allow_low_precision(reason="softmax recip as f32r"):
                            nc.vector.reciprocal(rc[0:1, :], O[64:65, :])
                        bc = norm.tile([P, 512], f32r, tag="bc")
                        if p == 3:
                            # PE broadcast: lower latency than the DRAM bounce
                            bcp = ps_m.tile([P, 512], f32, tag="mm")
                            nc.tensor.matmul(bcp[0:64, :], ones64[:],
                                             rc[0:1, :], start=True, stop=True)
                            nc.vector.tensor_copy(bc[0:64, :], bcp[0:64, :])
                        else:
                            dsc = drp.tile([1, 512], f32r, tag="dsc")
                            nc.sync.dma_start(dsc[:], rc[0:1, :])
                            nc.sync.dma_start(bc[0:64, :], _bcast_ap(dsc, 64))
                        nc.vector.tensor_mul(
                            o3d[p][64 * hs : 64 * hs + 64,
                                   16 * ih : 16 * ih + 16, 1 : 1 + W],
                            O[0:64, :].rearrange("p (a b) -> p a b", b=W),
                            bc[0:64, :].rearrange("p (a b) -> p a b", b=W),
                        )

            _dw3x3(nc, ps_m, od[3], o3d[3], dgo, 3, evac_act)

            # ---------------- final pointwise + store -----------------------
            accs = [ps_t.tile([P, 1024], f32, tag="T", name=f"opwacc{oc}")
                    for oc in range(2)]
            for kc in range(4):
                for oc in range(2):
                    for nh in range(2):
                        nc.tensor.matmul(
                            accs[oc][:, nh * 512 : (nh + 1) * 512],
                            opw[:, kc, oc * P : (oc + 1) * P],
                            od[kc][:, nh * 512 : (nh + 1) * 512],
                            start=(kc == 0), stop=(kc == 3),
                        )
            for oc in range(2):
                out_sb = persist.tile([P, N], f32, name=f"outsb{oc}")
                for nh in range(2):
                    ev = evac_act if oc == 0 else evac_dve
                    ev(accs[oc][:, nh * 512 : (nh + 1) * 512],
                       out_sb[:, nh * 512 : (nh + 1) * 512])
                    nc.sync.dma_start(
                        out_ap[oc * P : (oc + 1) * P, nh * 512 : (nh + 1) * 512],
                        out_sb[:, nh * 512 : (nh + 1) * 512],
                    )

    return nc


_NC_CACHE = {}
LAST_RESULTS = None


def _get_nc():
    if "nc" not in _NC_CACHE:
        _NC_CACHE["nc"] = _build_nc()
    return _NC_CACHE["nc"]


def _prep_weights(q_dw, q_pw, kv_dw, kv_pw, out_dw, out_pw):
    m = np.arange(INNER)
    perm = (m % D) * HEADS + (m // D)        # head-major -> original channel

    def pw_T(w):                              # [out, in] -> SBUF [128, in/128, out]
        wT = np.ascontiguousarray(w.T)        # [in, out]
        kchunks = wT.shape[0] // P
        return np.ascontiguousarray(
            wT.reshape(kchunks, P, wT.shape[1]).transpose(1, 0, 2)
        )

    qpw = pw_T(q_pw.reshape(INNER, C)[perm])
    kpw = pw_T(kv_pw.reshape(2 * INNER, C)[:INNER][perm])
    vpw = pw_T(kv_pw.reshape(2 * INNER, C)[INNER:][perm])
    opw = pw_T(out_pw.reshape(C, INNER)[:, perm])     # -> lhsT [128, 4, 256]

    qdw = q_dw.reshape(C, 9)
    xdw = kv_dw.reshape(C, 9)
    odw = out_dw.reshape(INNER, 9)[perm]

    dw9qx = np.stack([qdw[0:P], qdw[P:2 * P], xdw[0:P], xdw[P:2 * P]], axis=1)
    dw9o = np.stack([odw[0:P], odw[P:2 * P], odw[2 * P:3 * P], odw[3 * P:4 * P]],
                    axis=1)

    return {
        "ident": np.eye(P, dtype=np.float32),
        "dw9qx": np.ascontiguousarray(dw9qx),
        "dw9o": np.ascontiguousarray(dw9o),
        "ones64": np.ones((1, 64), np.float32),
        "qpw": qpw,
        "kpw": kpw,
        "vpw": vpw,
        "opw": opw,
    }


def kernel(q, x, q_dw, q_pw, kv_dw, kv_pw, out_dw, out_pw):
    global LAST_RESULTS
    q = np.asarray(q, np.float32)
    x = np.asarray(x, np.float32)
    weights = _prep_weights(
        np.asarray(q_dw, np.float32), np.asarray(q_pw, np.float32),
        np.asarray(kv_dw, np.float32), np.asarray(kv_pw, np.float32),
        np.asarray(out_dw, np.float32), np.asarray(out_pw, np.float32),
    )
    in_maps = []
    for b in range(N_CORES):
        qp = np.zeros((C, H, W + 2), np.float32)
        qp[:, :, 1 : 1 + W] = q[b].reshape(C, H, W)
        xp = np.zeros((C, H, W + 2), np.float32)
        xp[:, :, 1 : 1 + W] = x[b].reshape(C, H, W)
        m = {"q": qp.reshape(C, -1), "x": xp.reshape(C, -1)}
        m.update(weights)
        in_maps.append(m)

    nc = _get_nc()
    res = bass_utils.run_bass_kernel_spmd(nc, in_maps, core_ids=list(range(N_CORES)))
    LAST_RESULTS = res
    out = np.stack([res.results[b]["out"].reshape(C, H, W) for b in range(N_CORES)])
    return out.astype(np.float32)


# revision 20
# speedup vs baseline: 561.9072x; 1.0615x over previous
"""Trainium2 Bass kernel for nn_AttentionDe_lm (conv-projected multi-head attention).

Strategy: pure data-parallel over batch B=8 -> one batch element per NeuronCore.
Per core, everything is formulated as PE matmuls in a channels-on-partitions
layout [C, H*W]:

  - depthwise 3x3 convs   -> 9 PSUM-accumulated matmuls with diagonal weight
                             matrices; zero padding is realized by clipping the
                             per-tap input windows (center tap issued first so
                             the accumulation covers the whole tile)
  - pointwise 1x1 convs   -> plain matmuls (weights pre-transposed and
                             head-major-permuted on the host)
  - attention             -> computed transposed: T = S^T tiles [j, i] so that
                             QK^T needs no transposes at all; the two heads of a
                             partition chunk run concurrently in the PE array
                             via row tile_position packing; exp on ScalarE with
                             the attention scale fused in (inputs are tiny, so
                             no max-subtraction is needed); AV consumes E with V
                             extended by a ones column (M=65) so the softmax
                             denominators emerge from the same matmul chain
  - softmax normalization -> reciprocal + partition-broadcast (via a DRAM
                             bounce) + one fused multiply that writes the bf16
                             input of the output depthwise conv

Matmuls run in fp32r (full PE rate at N=512); attention probabilities and the
output-side depthwise conv run in bf16.
"""

import sys

sys.path.insert(0, "/opt/trn_rl_repo")

import numpy as np
import concourse.bass as bass
import concourse.tile as tile
from concourse import mybir, bass_utils
from concourse.vector_clock import ScopedClock, VectorClock

# ---------------------------------------------------------------------------
# TileContext adapted to a walrus build that allows at most ONE sync-wait per
# instruction: hoist extra waits onto EventSemaphore instructions, and replace
# the multi-wait final Drain with per-sem single-wait SP no-ops.
# ---------------------------------------------------------------------------

_ev_counter = [0]


class SplitDrainTileContext(tile.TileContext):
    def _split_multi_waits(self):
        f = self.nc.cur_f
        assert f is not None
        for bb in f.blocks[self.starting_block_idx :]:
            out = []
            changed = False
            for inst in list(bb.instructions):
                si = inst.sync_info
                if si is not None and len(si.on_wait) > 1:
                    changed = True
                    waits = list(si.on_wait)
                    for w in waits[:-1]:
                        _ev_counter[0] += 1
                        ev = mybir.InstEventSemaphore(name=f"IW-{_ev_counter[0]}")
                        ev.engine = inst.engine
                        ev.sync_info = mybir.SyncInfo(on_wait=[w], on_update=[])
                        self.nc.register_instruction(ev, overwrite=True)
                        out.append(ev)
                    inst.sync_info = mybir.SyncInfo(
                        on_wait=[waits[-1]], on_update=list(si.on_update)
                    )
                out.append(inst)
            if changed:
                bb.instructions = out

    def _drain_and_barrier(self, tick_clock, wait_clock):
        gvec = list(tick_clock.global_clock)
        nprocs = len(gvec)
        for p, t in enumerate(gvec):
            if t <= 0:
                continue
            vec = [0] * nprocs
            vec[p] = t
            ev = self.nc.sync.nop()
            wait_clock.add_sem_waits(ev.ins, ScopedClock({None: VectorClock(vec)}))
        self.nc.sync.drain()
        self.nc.all_engine_barrier()
        assert self.sems is not None
        popped = self.nc._tile_sem_poison_stack.pop()
        assert popped is self._sem_poison
        self.nc.clear_and_free_semaphores(list(self.sems.allocated().values()))
        self.nc.all_engine_barrier()
        self._split_multi_waits()


# ---------------------------------------------------------------------------
# Problem constants (hardcoded per the harness contract)
# ---------------------------------------------------------------------------

B, C, H, W = 8, 256, 32, 32
N = H * W                      # 1024 spatial positions
HEADS, D = 8, 64
INNER = HEADS * D              # 512
SCALE = D ** -0.5
P = 128
N_CORES = 8

f32 = mybir.dt.float32
f32r = mybir.dt.float32r
bf16 = mybir.dt.bfloat16
Exp = mybir.ActivationFunctionType.Exp

# center tap first: its full window makes start=True cover the whole psum tile
TAP_ORDER = [4, 0, 1, 2, 3, 5, 6, 7, 8]


def _bcast_ap(dram_tile, parts):
    """Partition-broadcast view of a [1, F] DRAM tile."""
    return bass.AP(
        tensor=dram_tile.tensor,
        offset=dram_tile.offset,
        ap=[[0, parts]] + list(dram_tile.ap[1:]),
    )


def _dw3x3(nc, ps_pool, dst_sb, src3d, diag, slot, evac):
    """Depthwise 3x3 (pad=1) on a width-padded [128, 32, 34] image via 9
    PSUM-accumulated diagonal matmuls. The center tap goes first so its full
    window makes start=True cover the whole psum tile; vertical padding is
    realized by clipping output rows.

    dst_sb: [128, 1024] SBUF tile; evac: copy fn (psum_ap, dst_ap) -> None.
    """
    for half in range(2):
        _dw3x3_half(nc, ps_pool, dst_sb, src3d, diag, slot, half, evac)


def _dw3x3_half(nc, ps_pool, dst_sb, src3d, diag, slot, half, evac):
    r0 = half * 16
    acc = ps_pool.tile([P, 16, W], f32, tag="mm")
    for i, t in enumerate(TAP_ORDER):
        oy, dx = t // 3 - 1, t % 3
        rs, re = max(r0, -oy), min(r0 + 16, H - oy)
        nc.tensor.matmul(
            acc[:, rs - r0 : re - r0, :],
            diag[:, slot, t, :],
            src3d[:, rs + oy : re + oy, dx : dx + W],
            start=(i == 0), stop=(i == 8),
        )
    evac(acc[:].rearrange("p a b -> p (a b)"),
         dst_sb[:, half * 512 : (half + 1) * 512])


def _build_nc():
    nc = bass.Bass("TRN2", target_bir_lowering=False, debug=False, enable_asserts=True)

    # Per-core inputs (one batch element) + replicated preprocessed weights.
    q_ap = nc.dram_tensor("q", (C, H * (W + 2)), f32r, kind="ExternalInput").ap()
    x_ap = nc.dram_tensor("x", (C, H * (W + 2)), f32r, kind="ExternalInput").ap()
    ident_ap = nc.dram_tensor("ident", (P, P), f32, kind="ExternalInput").ap()
    dw9qx_ap = nc.dram_tensor("dw9qx", (P, 4, 9), f32, kind="ExternalInput").ap()
    dw9o_ap = nc.dram_tensor("dw9o", (P, 4, 9), f32, kind="ExternalInput").ap()
    qpw_ap = nc.dram_tensor("qpw", (P, 2, INNER), f32r, kind="ExternalInput").ap()
    kpw_ap = nc.dram_tensor("kpw", (P, 2, INNER), f32r, kind="ExternalInput").ap()
    vpw_ap = nc.dram_tensor("vpw", (P, 2, INNER), f32r, kind="ExternalInput").ap()
    opw_ap = nc.dram_tensor("opw", (P, 4, C), f32r, kind="ExternalInput").ap()
    ones_ap = nc.dram_tensor("ones64", (1, 64), f32r, kind="ExternalInput").ap()
    out_ap = nc.dram_tensor("out", (C, N), f32, kind="ExternalOutput").ap()

    with SplitDrainTileContext(nc) as tc:
        with (
            tc.tile_pool(name="const", bufs=1) as const,
            tc.tile_pool(name="persist", bufs=1) as persist,
            tc.tile_pool(name="epool", bufs=10) as epool,
            tc.tile_pool(name="norm", bufs=3) as norm,
            tc.tile_pool(name="dram", bufs=4, space="DRAM") as drp,
            tc.tile_pool(name="ps_t", bufs=2, space="PSUM") as ps_t,
            tc.tile_pool(name="ps_o", bufs=2, space="PSUM") as ps_o,
            tc.tile_pool(name="ps_m", bufs=2, space="PSUM") as ps_m,
        ):
            # ---------------- constant weights ----------------
            # (depthwise diagonals are expanded on-device from compact
            #  [128, 4, 9] host arrays: diag = identity * w, per partition)
            ident = const.tile([P, P], f32)
            nc.sync.dma_start(ident[:], ident_ap[:])
            dw9qx = const.tile([P, 4, 9], f32)
            nc.sync.dma_start(dw9qx[:], dw9qx_ap[:])
            dw9o = const.tile([P, 4, 9], f32)
            nc.scalar.dma_start(dw9o[:], dw9o_ap[:])
            dgo = const.tile([P, 4, 9, P], bf16)
            qpw = const.tile([P, 2, INNER], f32r)
            nc.scalar.dma_start(qpw[:], qpw_ap[:])
            kpw = const.tile([P, 2, INNER], f32r)
            nc.scalar.dma_start(kpw[:], kpw_ap[:])
            vpw = const.tile([P, 2, INNER], f32r)
            nc.scalar.dma_start(vpw[:], vpw_ap[:])
            opw = const.tile([P, 4, C], f32r)
            nc.scalar.dma_start(opw[:], opw_ap[:])
            ones64 = const.tile([1, 64], f32r)
            nc.sync.dma_start(ones64[:], ones_ap[:])

            # persistent activations
            Q = [persist.tile([P, N], f32r, name=f"Q{i}") for i in range(4)]
            K = [persist.tile([P, N], f32r, name=f"K{i}") for i in range(4)]
            Vp = [persist.tile([P, HEADS, D + 1], bf16, name=f"Vp{i}")
                  for i in range(8)]
            o3d = [persist.tile([P, H, W + 2], bf16, name=f"o3d{i}") for i in range(4)]
            od = [persist.tile([P, N], f32r, name=f"od{i}") for i in range(4)]
            for ck in range(4):
                nc.scalar.memzero(o3d[ck][:])

            def evac_act(src, dst):
                nc.scalar.copy(dst, src)

            def evac_dve(src, dst):
                nc.vector.tensor_copy(dst, src)

            # ---------------- phase A: input dw convs + projections ---------
            with tc.tile_pool(name="phaseA", bufs=1) as pa:
                dgqx = pa.tile([P, 4, 9, P], f32r)
                for s in range(4):
                    for t in range(9):
                        nc.vector.tensor_scalar_mul(
                            dgqx[:, s, t, :], ident[:], dw9qx[:, s, t : t + 1]
                        )
                dma_engines = [nc.sync, nc.scalar, nc.gpsimd]
                di = 0
                rawmap = {}
                for src_ap, nm in ((x_ap, "xr"), (q_ap, "qr")):
                    for ck in range(2):
                        raw = pa.tile([P, H, W + 2], f32r, name=f"{nm}{ck}")
                        for hh in range(2):
                            dma_engines[di % 3].dma_start(
                                raw[64 * hh : 64 * hh + 64],
                                src_ap[ck * P + 64 * hh : ck * P + 64 * hh + 64, :]
                                .rearrange("p (a b) -> p a b", b=W + 2),
                            )
                            di += 1
                        rawmap[f"{nm}{ck}"] = raw
                raws = [rawmap["qr0"], rawmap["qr1"], rawmap["xr0"], rawmap["xr1"]]

                qd = [persist.tile([P, N], f32r, name=f"qd{i}") for i in range(2)]
                xd = [persist.tile([P, N], f32r, name=f"xd{i}") for i in range(2)]
                for s in (2, 3, 0, 1):              # x first: K/V unblock sooner
                    dst = qd[s] if s < 2 else xd[s - 2]
                    _dw3x3(nc, ps_m, dst, raws[s], dgqx, s, evac_act)

                def proj_qk_half(oc, which):
                    w_sb, dsrc, dst = ((qpw, qd, Q), (kpw, xd, K))[which]
                    for nh in range(2):
                        acc = ps_m.tile([P, 512], f32, tag="mm")
                        for kc in range(2):
                            nc.tensor.matmul(
                                acc[:], w_sb[:, kc, oc * P : (oc + 1) * P],
                                dsrc[kc][:, nh * 512 : (nh + 1) * 512],
                                start=(kc == 0), stop=(kc == 1),
                            )
                        nc.vector.tensor_copy(
                            dst[oc][:, nh * 512 : (nh + 1) * 512], acc[:]
                        )

                def proj_qk(oc):
                    for w_sb, dsrc, dst in ((qpw, qd, Q), (kpw, xd, K)):
                        for nh in range(2):
                            acc = ps_m.tile([P, 512], f32, tag="mm")
                            for kc in range(2):
                                nc.tensor.matmul(
                                    acc[:], w_sb[:, kc, oc * P : (oc + 1) * P],
                                    dsrc[kc][:, nh * 512 : (nh + 1) * 512],
                                    start=(kc == 0), stop=(kc == 1),
                                )
                            nc.vector.tensor_copy(
                                dst[oc][:, nh * 512 : (nh + 1) * 512], acc[:]
                            )

                proj_qk(0)
                # V transposed: V^T[j, v_ch]; per j-chunk [128, 8, 65]
                # (per head: 64 value dims + ones column for softmax sums).
                # Emission is deferred into pair 0's jc loop so the first QK
                # tiles reach the PE sooner.
                for jc in range(8):
                    nc.vector.memset(Vp[jc][:], 1.0)

                for jc in range(8):
                    acc = ps_m.tile([P, 512], f32, tag="mm")
                    for kc in range(2):
                        nc.tensor.matmul(
                            acc[:], xd[kc][:, jc * P : (jc + 1) * P],
                            vpw[:, kc, :], start=(kc == 0), stop=(kc == 1),
                        )
                    nc.vector.tensor_copy(
                        Vp[jc][:, :, 0:D],
                        acc[:].rearrange("p (h d) -> p h d", d=D),
                    )

            for s in range(4):
                for t in range(9):
                    nc.vector.tensor_scalar_mul(
                        dgo[:, s, t, :], ident[:], dw9o[:, s, t : t + 1]
                    )

            # ---------------- attention + output path ----------------------
            for p in range(4):                       # head pair (= chunk)
                for ih in range(2):                  # query half (i block)
                    Otiles = [ps_o.tile([P, 512], f32, tag="O", name=f"O{p}{ih}{k}")
                              for k in range(2)]
                    for jc in range(8):
                        T = ps_t.tile([P, 1024], f32, tag="T")
                        nc.tensor.matmul(
                            T[:, 0:512],
                            K[p][0:64, jc * P : (jc + 1) * P],
                            Q[p][0:64, ih * 512 : (ih + 1) * 512],
                            start=True, stop=True, tile_position=(0, 0),
                        )
                        nc.tensor.matmul(
                            T[:, 512:1024],
                            K[p][64:128, jc * P : (jc + 1) * P],
                            Q[p][64:128, ih * 512 : (ih + 1) * 512],
                            start=True, stop=True, tile_position=(64, 0),
                        )
                        E = epool.tile([P, 1024], bf16, tag="E")
                        nc.scalar.activation(E[:], T[:], Exp, scale=SCALE)
                        for hs in range(2):
                            nc.tensor.matmul(
                                Otiles[hs][0:65, :],
                                Vp[jc][:, 2 * p + hs, :],
                                E[:, hs * 512 : (hs + 1) * 512],
                                start=(jc == 0), stop=(jc == 7),
                            )
                        if p >= 1 and jc == 3:
                            _dw3x3_half(nc, ps_m, od[p - 1], o3d[p - 1],
                                        dgo, p - 1, ih, evac_dve)
                        if p < 3 and jc == 6:
                            proj_qk_half(p + 1, ih)
                    for hs in range(2):              # head within the pair
                        O = Otiles[hs]
                        # softmax denominators sit in row 64; normalize rows
                        # 0..63 into the bf16 out-conv input.
                        rc = norm.tile([P, 512], f32r, tag="rc")
                        with nc.allow_low_precision(reason="softmax recip as f32r"):
                            nc.vector.reciprocal(rc[0:1, :], O[64:65, :])
                        bc = norm.tile([P, 512], f32r, tag="bc")
                        if p == 3:
                            # PE broadcast: lower latency than the DRAM bounce
                            bcp = ps_m.tile([P, 512], f32, tag="mm")
                            nc.tensor.matmul(bcp[0:64, :], ones64[:],
                                             rc[0:1, :], start=True, stop=True)
                            nc.vector.tensor_copy(bc[0:64, :], bcp[0:64, :])
                        else:
                            dsc = drp.tile([1, 512], f32r, tag="dsc")
                            nc.sync.dma_start(dsc[:], rc[0:1, :])
                            nc.sync.dma_start(bc[0:64, :], _bcast_ap(dsc, 64))
                        nc.vector.tensor_mul(
                            o3d[p][64 * hs : 64 * hs + 64,
                                   16 * ih : 16 * ih + 16, 1 : 1 + W],
                            O[0:64, :].rearrange("p (a b) -> p a b", b=W),
                            bc[0:64, :].rearrange("p (a b) -> p a b", b=W),
                        )

            _dw3x3(nc, ps_m, od[3], o3d[3], dgo, 3, evac_act)

            # ---------------- final pointwise + store -----------------------
            accs = [ps_t.tile([P, 1024], f32, tag="T", name=f"opwacc{oc}")
                    for oc in range(2)]
            for kc in range(4):
                for oc in range(2):
                    for nh in range(2):
                        nc.tensor.matmul(
                            accs[oc][:, nh * 512 : (nh + 1) * 512],
                            opw[:, kc, oc * P : (oc + 1) * P],
                            od[kc][:, nh * 512 : (nh + 1) * 512],
                            start=(kc == 0), stop=(kc == 3),
                        )
            for oc in range(2):
                out_sb = persist.tile([P, N], f32, name=f"outsb{oc}")
                for nh in range(2):
                    ev = evac_act if oc == 0 else evac_dve
                    ev(accs[oc][:, nh * 512 : (nh + 1) * 512],
                       out_sb[:, nh * 512 : (nh + 1) * 512])
                    nc.sync.dma_start(
                        out_ap[oc * P : (oc + 1) * P, nh * 512 : (nh + 1) * 512],
                        out_sb[:, nh * 512 : (nh + 1) * 512],
                    )

    return nc


_NC_CACHE = {}
LAST_RESULTS = None


def _get_nc():
    if "nc" not in _NC_CACHE:
        _NC_CACHE["nc"] = _build_nc()
    return _NC_CACHE["nc"]


def _prep_weights(q_dw, q_pw, kv_dw, kv_pw, out_dw, out_pw):
    m = np.arange(INNER)
    perm = (m % D) * HEADS + (m // D)        # head-major -> original channel

    def pw_T(w):                              # [out, in] -> SBUF [128, in/128, out]
        wT = np.ascontiguousarray(w.T)        # [in, out]
        kchunks = wT.shape[0] // P
        return np.ascontiguousarray(
            wT.reshape(kchunks, P, wT.shape[1]).transpose(1, 0, 2)
        )

    qpw = pw_T(q_pw.reshape(INNER, C)[perm])
    kpw = pw_T(kv_pw.reshape(2 * INNER, C)[:INNER][perm])
    vpw = pw_T(kv_pw.reshape(2 * INNER, C)[INNER:][perm])
    opw = pw_T(out_pw.reshape(C, INNER)[:, perm])     # -> lhsT [128, 4, 256]

    qdw = q_dw.reshape(C, 9)
    xdw = kv_dw.reshape(C, 9)
    odw = out_dw.reshape(INNER, 9)[perm]

    dw9qx = np.stack([qdw[0:P], qdw[P:2 * P], xdw[0:P], xdw[P:2 * P]], axis=1)
    dw9o = np.stack([odw[0:P], odw[P:2 * P], odw[2 * P:3 * P], odw[3 * P:4 * P]],
                    axis=1)

    return {
        "ident": np.eye(P, dtype=np.float32),
        "dw9qx": np.ascontiguousarray(dw9qx),
        "dw9o": np.ascontiguousarray(dw9o),
        "ones64": np.ones((1, 64), np.float32),
        "qpw": qpw,
        "kpw": kpw,
        "vpw": vpw,
        "opw": opw,
    }


def kernel(q, x, q_dw, q_pw, kv_dw, kv_pw, out_dw, out_pw):
    global LAST_RESULTS
    q = np.asarray(q, np.float32)
    x = np.asarray(x, np.float32)
    weights = _prep_weights(
        np.asarray(q_dw, np.float32), np.asarray(q_pw, np.float32),
        np.asarray(kv_dw, np.float32), np.asarray(kv_pw, np.float32),
        np.asarray(out_dw, np.float32), np.asarray(out_pw, np.float32),
    )
    in_maps = []
    for b in range(N_CORES):
        qp = np.zeros((C, H, W + 2), np.float32)
        qp[:, :, 1 : 1 + W] = q[b].reshape(C, H, W)
        xp = np.zeros((C, H, W + 2), np.float32)
        xp[:, :, 1 : 1 + W] = x[b].reshape(C, H, W)
        m = {"q": qp.reshape(C, -1), "x": xp.reshape(C, -1)}
        m.update(weights)
        in_maps.append(m)

    nc = _get_nc()
    res = bass_utils.run_bass_kernel_spmd(nc, in_maps, core_ids=list(range(N_CORES)))
    LAST_RESULTS = res
    out = np.stack([res.results[b]["out"].reshape(C, H, W) for b in range(N_CORES)])
    return out.astype(np.float32)
